# revision 16
# baseline (speedup 1.0000x reference)
"""Trainium2 Bass kernel for nn_ConcatRelationModule (gnn_message_passing).

Strategy: data-parallel over the edge dimension E across 8 NeuronCores.
 - Sharding/prep (host, untimed): edges split contiguously, 32768 per core.
   The per-edge head rows fwd[gold_heads] are materialized during sharding
   (the "gathered for the head indices" option of the sharding hint) and the
   modifier rows bwd[e+1] are a contiguous slice; both are shipped
   pre-transposed as fp16 [128, 32768] per core (~17 MB/core, vs. the
   replicated 128 MB fwd table a dma_gather design needs).
 - Device kernel per 512-edge tile: 7 accumulating fp16 matmuls on the PE
   (cat->headfov/modfov, h->h2, h2->scores), tanh/bias on ScalarE, PE
   transpose of scores to edge-major, hinge (gold vs best-wrong label) on
   VectorE. lerrs accumulate in SBUF and are transposed out at the end.
"""
import sys

sys.path.insert(0, "/opt/trn_rl_repo")

import numpy as np

import concourse.bass as bass
import concourse.bacc as bacc
import concourse.mybir as mybir
import concourse.tile as tile
from concourse.bass_utils import run_bass_kernel_spmd
from concourse.masks import make_identity

F32 = mybir.dt.float32
F32R = mybir.dt.float32r
F16 = mybir.dt.float16

N = 262144
L = 128
H = 128
H2 = 128
R = 64
E = N - 1
NCORES = 8
EPC = N // NCORES            # edges per core (the very last edge is padding)
NT = EPC // 512              # 64 tiles of 512 edges
NB = EPC // 128              # 256 blocks of 128 edges


def build_kernel():
    nc = bacc.Bacc("TRN2", target_bir_lowering=False, debug=False)

    # cat2 rows 0:128 = fwd[heads].T, 128:256 = bwd[mods].T, row 256 = rels
    # (fp16, flattened [128, NB] partition-major so the DMA is 512B/partition)
    cat2_d = nc.declare_dram_parameter("cat2", [2 * L + 1, EPC], F16, isOutput=False)
    # w16 rows: 0:256 WFOH, 256:512 WFOM, 512:768 rhid2, 768:896 rout (cols 0:64)
    w16_d = nc.declare_dram_parameter("w16", [896, 128], F16, isOutput=False)
    # b32: 0:128 bcat[:128], 128:256 bcat[128:], 256:384 rhid2Bias, 384:448 routBias
    b32_d = nc.declare_dram_parameter("b32", [512], F32, isOutput=False)

    lerr_d = nc.declare_dram_parameter("lerr", [EPC], F32, isOutput=True)
    lerr_v = lerr_d[:].rearrange("(b p) -> b p", p=128)

    with tile.TileContext(nc) as tc:
        with (
            tc.tile_pool(name="const", bufs=1) as cp,
            tc.tile_pool(name="inp", bufs=4) as gp,
            tc.tile_pool(name="work", bufs=2) as wp,
            tc.tile_pool(name="ps", bufs=1, space="PSUM") as pp,
            tc.tile_pool(name="ps2", bufs=2, space="PSUM") as pp2,
        ):
            # ---- constants ----
            ident = cp.tile([128, 128], F32, tag="ident")
            make_identity(nc, ident[:])


            wfoh_f = cp.tile([128, H], F16, tag="wfoh_f")
            wfoh_b = cp.tile([128, H], F16, tag="wfoh_b")
            wfom_f = cp.tile([128, H], F16, tag="wfom_f")
            wfom_b = cp.tile([128, H], F16, tag="wfom_b")
            rh2_a = cp.tile([128, H2], F16, tag="rh2_a")
            rh2_b = cp.tile([128, H2], F16, tag="rh2_b")
            rout_t = cp.tile([128, R], F16, tag="rout_t")
            nc.sync.dma_start(out=wfoh_f[:], in_=w16_d[0:128, :])
            nc.sync.dma_start(out=wfoh_b[:], in_=w16_d[128:256, :])
            nc.sync.dma_start(out=wfom_f[:], in_=w16_d[256:384, :])
            nc.sync.dma_start(out=wfom_b[:], in_=w16_d[384:512, :])
            nc.sync.dma_start(out=rh2_a[:], in_=w16_d[512:640, :])
            nc.sync.dma_start(out=rh2_b[:], in_=w16_d[640:768, :])
            nc.sync.dma_start(out=rout_t[:], in_=w16_d[768:896, 0:R])

            bias_h = cp.tile([128, 1], F32, tag="bias_h")
            bias_m = cp.tile([128, 1], F32, tag="bias_m")
            bias_2 = cp.tile([128, 1], F32, tag="bias_2")
            bias_r = cp.tile([64, 1], F32, tag="bias_r")
            nc.sync.dma_start(out=bias_h[:], in_=b32_d[0:128].rearrange("(p o) -> p o", o=1))
            nc.sync.dma_start(out=bias_m[:], in_=b32_d[128:256].rearrange("(p o) -> p o", o=1))
            nc.sync.dma_start(out=bias_2[:], in_=b32_d[256:384].rearrange("(p o) -> p o", o=1))
            nc.sync.dma_start(out=bias_r[:], in_=b32_d[384:448].rearrange("(p o) -> p o", o=1))

            iota_t = cp.tile([128, 4 * R], F32, tag="iota")
            nc.gpsimd.iota(
                out=iota_t[:].rearrange("p (j r) -> p j r", r=R),
                pattern=[[0, 4], [1, R]],
                channel_multiplier=0,
                allow_small_or_imprecise_dtypes=True,
            )

            rels_h = cp.tile([128, NB], F16, tag="rels_h")
            nc.sync.dma_start(
                out=rels_h[:],
                in_=cat2_d[2 * L, :].rearrange("(p b) -> p b", p=128),
            )
            rels_sb = cp.tile([128, NB], F32, tag="rels_sb")
            nc.vector.tensor_copy(out=rels_sb[:], in_=rels_h[:])

            lerr_acc = cp.tile([128, NB], F32, tag="lerr_acc")

            # ---- main pipeline ----
            for t in range(NT):
                fwdT_t = gp.tile([128, 512], F16, tag="fwdT_t")
                nc.sync.dma_start(out=fwdT_t[:], in_=cat2_d[0:128, t * 512:(t + 1) * 512])
                bwdT_t = gp.tile([128, 512], F16, tag="bwdT_t")
                nc.sync.dma_start(out=bwdT_t[:], in_=cat2_d[128:256, t * 512:(t + 1) * 512])

                fov = pp.tile([128, 512], F32, tag="fov")
                nc.tensor.matmul(out=fov[:], lhsT=wfoh_f[:], rhs=fwdT_t[:],
                                 start=True, stop=False)
                nc.tensor.matmul(out=fov[:], lhsT=wfoh_b[:], rhs=bwdT_t[:],
                                 start=False, stop=True)
                h1 = wp.tile([128, 512], F16, tag="h1")
                nc.scalar.activation(
                    out=h1[:], in_=fov[:],
                    func=mybir.ActivationFunctionType.Tanh,
                    bias=bias_h[:, 0:1],
                )

                mov = pp.tile([128, 512], F32, tag="mov")
                nc.tensor.matmul(out=mov[:], lhsT=wfom_f[:], rhs=fwdT_t[:],
                                 start=True, stop=False)
                nc.tensor.matmul(out=mov[:], lhsT=wfom_b[:], rhs=bwdT_t[:],
                                 start=False, stop=True)
                h1m = wp.tile([128, 512], F16, tag="h1m")
                nc.scalar.activation(
                    out=h1m[:], in_=mov[:],
                    func=mybir.ActivationFunctionType.Tanh,
                    bias=bias_m[:, 0:1],
                )

                h2p = pp.tile([128, 512], F32, tag="h2p")
                nc.tensor.matmul(out=h2p[:], lhsT=rh2_a[:], rhs=h1[:],
                                 start=True, stop=False)
                nc.tensor.matmul(out=h2p[:], lhsT=rh2_b[:], rhs=h1m[:],
                                 start=False, stop=True)
                h2s = wp.tile([128, 512], F16, tag="h2s")
                nc.scalar.activation(
                    out=h2s[:], in_=h2p[:],
                    func=mybir.ActivationFunctionType.Tanh,
                    bias=bias_2[:, 0:1],
                )

                scp = pp2.tile([64, 512], F32, tag="scp")
                nc.tensor.matmul(out=scp[:], lhsT=rout_t[:], rhs=h2s[:],
                                 start=True, stop=True)
                ssb = wp.tile([64, 512], F32, tag="ssb")
                nc.scalar.activation(
                    out=ssb[:], in_=scp[:],
                    func=mybir.ActivationFunctionType.Identity,
                    bias=bias_r[:, 0:1],
                )

                # scores back to [edge, label] layout
                stp = pp2.tile([128, 4 * R], F32, tag="stp")
                for k in range(4):
                    nc.tensor.transpose(
                        out=stp[:, k * R:(k + 1) * R],
                        in_=ssb[:, k * 128:(k + 1) * 128],
                        identity=ident[0:64, 0:64],
                    )
                st3 = stp[:].rearrange("p (j r) -> p j r", r=R)

                # hinge on VectorE
                relx = rels_sb[:, 4 * t:4 * t + 4].to_broadcast([128, 4, R])
                mask = wp.tile([128, 4 * R], F32, tag="mask")
                nc.vector.tensor_tensor(
                    out=mask[:].rearrange("p (j r) -> p j r", r=R),
                    in0=iota_t[:].rearrange("p (j r) -> p j r", r=R),
                    in1=relx,
                    op=mybir.AluOpType.is_equal,
                )
                m3 = mask[:].rearrange("p (j r) -> p j r", r=R)
                gmul = wp.tile([128, 4 * R], F32, tag="gmul")
                nc.vector.tensor_tensor(
                    out=gmul[:].rearrange("p (j r) -> p j r", r=R),
                    in0=st3, in1=m3, op=mybir.AluOpType.mult,
                )
                gold = wp.tile([128, 4], F32, tag="gold")
                nc.vector.reduce_sum(
                    out=gold[:], in_=gmul[:].rearrange("p (j r) -> p j r", r=R),
                    axis=mybir.AxisListType.X,
                )
                wm = wp.tile([128, 4 * R], F32, tag="wm")
                nc.vector.scalar_tensor_tensor(
                    out=wm[:].rearrange("p (j r) -> p j r", r=R),
                    in0=m3, scalar=-1e30, in1=st3,
                    op0=mybir.AluOpType.mult, op1=mybir.AluOpType.add,
                )
                wrong = wp.tile([128, 4], F32, tag="wrong")
                nc.vector.reduce_max(
                    out=wrong[:], in_=wm[:].rearrange("p (j r) -> p j r", r=R),
                    axis=mybir.AxisListType.X,
                )
                dtile = wp.tile([128, 4], F32, tag="dtile")
                nc.vector.tensor_tensor(
                    out=dtile[:], in0=wrong[:], in1=gold[:],
                    op=mybir.AluOpType.subtract,
                )
                nc.vector.scalar_tensor_tensor(
                    out=lerr_acc[:, 4 * t:4 * t + 4],
                    in0=dtile[:], scalar=-1.0, in1=dtile[:],
                    op0=mybir.AluOpType.is_gt, op1=mybir.AluOpType.mult,
                )

            # ---- write out lerrs (transpose to edge-major) ----
            for a in range(0, NB, 128):
                otp = pp2.tile([128, 128], F32, tag="stp")
                nc.tensor.transpose(
                    out=otp[:], in_=lerr_acc[:, a:a + 128], identity=ident[:],
                )
                osb = wp.tile([128, 128], F32, tag="osb")
                nc.scalar.copy(out=osb[:], in_=otp[:])
                nc.sync.dma_start(out=lerr_v[a:a + 128, :], in_=osb[:])

    nc.compile()
    return nc


_NC_CACHE = {}


def _get_nc():
    if "nc" not in _NC_CACHE:
        _NC_CACHE["nc"] = build_kernel()
    return _NC_CACHE["nc"]


def make_weights(WFOH, WFOM, rcatBias, rhid2Layer, rhid2Bias, routLayer, routBias):
    w16 = np.zeros((896, 128), np.float16)
    w16[0:256] = np.asarray(WFOH, np.float16)
    w16[256:512] = np.asarray(WFOM, np.float16)
    w16[512:768] = np.asarray(rhid2Layer, np.float16)
    w16[768:896, 0:R] = np.asarray(routLayer, np.float16)
    b32 = np.zeros(512, np.float32)
    b32[0:256] = np.asarray(rcatBias, np.float32).reshape(-1)
    b32[256:384] = np.asarray(rhid2Bias, np.float32).reshape(-1)
    b32[384:448] = np.asarray(routBias, np.float32).reshape(-1)
    return dict(w16=w16, b32=b32)


def prepare_core_inputs(fwd, bwd, gold_heads, gold_rels, weights):
    """Shard edges contiguously; gather head rows / slice modifier rows.

    Core c owns edges [c*EPC, (c+1)*EPC); the single extra edge at the end
    (global index E) is padding with zero inputs.
    """
    fwd16 = np.asarray(fwd, dtype=np.float16)
    bwd16 = np.asarray(bwd, dtype=np.float16)
    heads = np.asarray(gold_heads, dtype=np.int64)
    rels = np.asarray(gold_rels, dtype=np.int64)

    in_maps = []
    for c in range(NCORES):
        lo, hi = c * EPC, (c + 1) * EPC
        if hi <= E:
            h_c = heads[lo:hi]
            fwd_rows = fwd16[h_c]                     # [EPC, L]
            bwd_rows = bwd16[lo + 1:hi + 1]           # contiguous modifiers
            r_c = rels[lo:hi]
        else:                                         # last core: pad 1 edge
            h_c = heads[lo:E]
            fwd_rows = np.zeros((EPC, L), np.float16)
            fwd_rows[:E - lo] = fwd16[h_c]
            bwd_rows = np.zeros((EPC, L), np.float16)
            bwd_rows[:E - lo] = bwd16[lo + 1:E + 1]
            r_c = np.zeros(EPC, np.int64)
            r_c[:E - lo] = rels[lo:E]

        cat2 = np.empty((2 * L + 1, EPC), np.float16)
        cat2[0:L] = fwd_rows.T
        cat2[L:2 * L] = bwd_rows.T
        # rels flattened from [128, NB] partition-major layout
        cat2[2 * L] = np.ascontiguousarray(
            r_c.astype(np.float16).reshape(NB, 128).T).reshape(-1)

        in_maps.append(dict(cat2=cat2, **weights))
    return in_maps


def assemble_output(results):
    lerr_full = np.empty(NCORES * EPC, dtype=np.float32)
    for c in range(NCORES):
        lerr_full[c * EPC:(c + 1) * EPC] = np.asarray(
            results[c]["lerr"], dtype=np.float32)
    return lerr_full[:E]


def kernel(fwd, bwd, gold_heads, gold_rels, WFOH, WFOM, rhidBias, rcatBias,
           rhid2Layer, rhid2Bias, routLayer, routBias):
    nc = _get_nc()
    weights = make_weights(WFOH, WFOM, rcatBias, rhid2Layer, rhid2Bias,
                           routLayer, routBias)
    in_maps = prepare_core_inputs(fwd, bwd, gold_heads, gold_rels, weights)
    res = run_bass_kernel_spmd(nc, in_maps, list(range(NCORES)))
    return assemble_output(res.results)
